# revision 26
# baseline (speedup 1.0000x reference)
"""CMDNet detector kernel for 8 Trainium2 NeuronCores.

Contract: kernel(**inputs) takes the FULL inputs of reference.setup_inputs()
(yt [16384,64], Ht [16384,64,32], sigmat0 [16384], m [4], alpha [4],
taui [65], delta [64]) and returns (ft [16384,32,4], xt [16384,32]) like
reference.reference().

Strategy (pure data parallel over batch):
 - shard batch over 8 cores (2048 samples each), identical SPMD program
 - per core: 2 groups x (128 partitions x 8 slots) sample layout
 - precompute per-sample HH = Ht^T Ht and yH = yt^T Ht on the TensorEngine
   via block-diagonal packing of the augmented matrix [Ht | yt] (2 samples
   per 128x66 matmul); extract PSUM -> per-sample SBUF layout with DMAs
 - 64 gradient-descent iterations run on Vector/Scalar engines in fp16
   (validated: end-to-end fp16 error ~5e-4 vs fp32 reference)
 - tiny per-iteration scalars (taui/delta + alpha uniformity) are read on
   the host and baked into the program as immediates
"""

import sys

sys.path.insert(0, "/opt/trn_rl_repo")

import numpy as np

import concourse.bass as bass  # noqa: F401  (bass types used implicitly)
import concourse.bacc as bacc
import concourse.mybir as mybir
import concourse.tile as tile
from concourse import bass_utils
from concourse.dve_ops import RECIPROCAL_APPROX_FAST, RECIP_APPROX_FAST_CONSTS

f16 = mybir.dt.float16
f32 = mybir.dt.float32
AL = mybir.AluOpType
ACTF = mybir.ActivationFunctionType

B, NR, NT, M, NITER = 16384, 64, 32, 4, 64
NCORES = 8
BS = B // NCORES          # samples per core
P = 128                   # SBUF partitions
NGROUP = 2                # groups per core
GS = BS // NGROUP         # samples per group (1024)
NS = GS // P              # slots per partition per group (8)
ST = NT * M               # state elems per sample (128)
HHW = NT * NT             # 1024
AUGW = NT + 1             # 33  (Ht cols + yt col)
PAIRW = 2 * AUGW          # 66  (block-diag pair width)
PAIRS = BS // 2           # 1024 pairs per core
PBLK = 16                 # pairs per input staging block
NBLK = PAIRS // PBLK      # 64 blocks
PPT = 8                   # pairs per PSUM tile (two banks, 128-elem pair pitch)

_CACHE = {}

# experiment flags (timing A/B via TimelineSim)
X_WHALE_UNIT = False   # whale mul with unit-stride in1 (wrong results; timing only)
X_SKIP_PRE = False     # skip gram precompute (wrong results; timing only)
X_GPSIMD = True        # offload exy/gq to gpsimd
X_GPSIMD2 = False      # also offload red2 + ed to gpsimd (model: worse)
X_GPSIMD3 = False      # t2 STT on gpsimd: opcode invalid on HW
X_G16 = True           # keep G state in fp16 (2x-mode updates)
X_GPSIMD4 = True       # z on gpsimd as two tensor_tensor ops
X_SKIP_EXTRACT = False # skip psum->stage extraction dmas (timing only)
X_SKIP_MM = False      # skip gram matmuls (timing only)
X_SKIP_INDMA = False   # skip ht/yt input dmas (timing only)


def _recip(nc, out_ap, in_ap):
    nc.vector._custom_dve(
        RECIPROCAL_APPROX_FAST,
        out=out_ap,
        in0=in_ap,
        s0=RECIP_APPROX_FAST_CONSTS["s0"],
        s1=RECIP_APPROX_FAST_CONSTS["s1"],
        imm2=RECIP_APPROX_FAST_CONSTS["imm2"],
    )


def _build(scalars):
    """Build + compile the per-core SPMD program. `scalars` carries the host
    -read values baked into the instruction stream."""
    m_vals, s_sm, ta_abs, d_vals, s_fin, la_term = scalars

    nc = bacc.Bacc("TRN2", target_bir_lowering=False, debug=False,
                   num_devices=NCORES)
    ht_d = nc.dram_tensor("ht", [BS, NR, NT], f32, kind="ExternalInput").ap()
    yt_d = nc.dram_tensor("yt", [BS, NR], f32, kind="ExternalInput").ap()
    sg_d = nc.dram_tensor("sg", [BS], f32, kind="ExternalInput").ap()
    ft_d = nc.dram_tensor("ft", [BS, NT, M], f32, kind="ExternalOutput").ap()
    xt_d = nc.dram_tensor("xt", [BS, NT], f32, kind="ExternalOutput").ap()

    ht_pairs = ht_d.rearrange("(k s) r i -> k s r i", s=2)
    yt_pairs = yt_d.rearrange("(k s) r -> k s r", s=2)

    with tile.TileContext(nc) as tc:
        with (
            tc.tile_pool(name="pers", bufs=1) as pp,
            tc.tile_pool(name="psum", bufs=4, space="PSUM") as psp,
            tc.tile_pool(name="pscopy", bufs=2) as scp,
        ):
            # ---------------- shared constants ----------------
            m16 = pp.tile([P, NS * ST], f16, tag="m16", name="m16")
            for mm in range(M):
                v = m16[:].rearrange("p (r m) -> p r m", m=M)[:, :, mm:mm + 1]
                nc.vector.memset(v, float(m_vals[mm]))

            sgf = pp.tile([P, NGROUP * NS], f32, tag="sgf", name="sgf")
            # sample (g, bi, sl, s) -> partition 64*s+bi, col g*NS+sl
            sg_v = sg_d.rearrange("(g bi sl s) -> g s bi sl", g=NGROUP, sl=NS, s=2)
            for g in range(NGROUP):
                for s in range(2):
                    nc.sync.dma_start(
                        sgf[64 * s:64 * (s + 1), g * NS:(g + 1) * NS],
                        sg_v[g, s],
                    )
            sig2f = pp.tile([P, NGROUP * NS], f32, tag="sig2f", name="sig2f")
            nc.scalar.activation(sig2f[:], sgf[:], ACTF.Square)

            # per-group persistent tiles
            g_t = {}
            for g in range(NGROUP):
                g_t[g] = dict(
                    hht=pp.tile([P, NS * HHW], f16, tag=f"hht{g}", name=f"hht{g}"),
                    yh=pp.tile([P, NS * NT], f32, tag=f"yh{g}", name=f"yh{g}"),
                    s2t=pp.tile([P, NS * ST], f16, tag=f"s2t{g}", name=f"s2t{g}"),
                    Ga=pp.tile([P, NS * ST], f16 if X_G16 else f32,
                                tag=f"Ga{g}", name=f"Ga{g}"),
                    Gb=pp.tile([P, NS * ST], f16 if X_G16 else f32,
                                tag=f"Gb{g}", name=f"Gb{g}"),
                    e16=pp.tile([P, NS * ST], f16, tag=f"e16{g}", name=f"e16{g}"),
                    v16=pp.tile([P, NS * ST], f16, tag=f"v16{g}", name=f"v16{g}"),
                    em=pp.tile([P, NS * ST], f16, tag=f"em{g}", name=f"em{g}"),
                    exy=pp.tile([P, NS * ST], f16, tag=f"exy{g}", name=f"exy{g}"),
                    ed=pp.tile([P, NS * ST], f16, tag=f"ed{g}", name=f"ed{g}"),
                    gq=pp.tile([P, NS * ST], f16, tag=f"gq{g}", name=f"gq{g}"),
                    z=pp.tile([P, NS * ST], f16, tag=f"z{g}", name=f"z{g}"),
                    t2=pp.tile([P, NS * ST], f16, tag=f"t2{g}", name=f"t2{g}"),
                    sa=pp.tile([P, NS * NT * 2], f16, tag=f"sa{g}", name=f"sa{g}"),
                    sm=pp.tile([P, NS * NT * 2], f16, tag=f"sm{g}", name=f"sm{g}"),
                    S=pp.tile([P, NS * NT], f32, tag=f"S{g}", name=f"S{g}"),
                    w=pp.tile([P, NS * NT], f16, tag=f"w{g}", name=f"w{g}"),
                    R=pp.tile([P, NS * NT], f32, tag=f"R{g}", name=f"R{g}"),
                    xt16=pp.tile([P, NS * NT], f16, tag=f"xt16{g}", name=f"xt16{g}"),
                    xhh=pp.tile([P, NS * NT], f32, tag=f"xhh{g}", name=f"xhh{g}"),
                    pr=pp.tile([P, NS * NT], f32, tag=f"pr{g}", name=f"pr{g}"),
                    q2=pp.tile([P, NS * NT], f16, tag=f"q2{g}", name=f"q2{g}"),
                )
            # whale scratch shared between groups (SBUF budget)
            tmp = pp.tile([P, NS * HHW], f16, tag="tmp", name="tmp")
            r1 = pp.tile([P, NS * NT * 16], f16, tag="r1", name="r1")
            r2 = pp.tile([P, NS * NT * 8], f16, tag="r2", name="r2")
            r3 = pp.tile([P, NS * NT * 4], f16, tag="r3", name="r3")
            r4 = pp.tile([P, NS * NT * 2], f16, tag="r4", name="r4")

            # block staging tiles for gram inputs (block-diagonal [Ht|y])
            Tb = [pp.tile([P, PBLK * PAIRW], f32, tag=f"Tb{i}", name=f"Tb{i}") for i in range(2)]
            nc.vector.memset(Tb[0][:], 0.0)
            nc.vector.memset(Tb[1][:], 0.0)
            T16b = [pp.tile([P, PBLK * PAIRW], f16, tag=f"T16b{i}", name=f"T16b{i}") for i in range(2)]

            # ---------------- per-group precompute + iterations -----------
            def precompute(g):
                stage = pp.tile([P, NS * AUGW * NT], f32, tag="stage", name="stage")
                blk0 = g * (NBLK // NGROUP)
                for blk in range(blk0, blk0 + NBLK // NGROUP):
                    T = Tb[blk % 2]
                    T16 = T16b[blk % 2]
                    # load [Ht | yt] diag blocks for 16 pairs
                    for s in range(2) if not X_SKIP_INDMA else []:
                        dst = T[64 * s:64 * (s + 1), :].rearrange(
                            "p (k c) -> p k c", c=PAIRW)
                        nc.sync.dma_start(
                            dst[:, :, AUGW * s: AUGW * s + NT],
                            ht_pairs[blk * PBLK:(blk + 1) * PBLK, s]
                            .transpose([1, 0, 2]),
                        )
                        nc.sync.dma_start(
                            dst[:, :, AUGW * s + NT: AUGW * s + NT + 1],
                            yt_pairs[blk * PBLK:(blk + 1) * PBLK, s]
                            .transpose([1, 0]).unsqueeze(2),
                        )
                    nc.vector.tensor_copy(T16[:], T[:])
                    for pt in range(PBLK // PPT) if not X_SKIP_MM else []:
                        ps = psp.tile([P, PPT * 128], f32, tag="gram", name="gram")
                        for k in range(PPT):
                            pr_ = pt * PPT + k
                            lhs = T16[:, PAIRW * pr_: PAIRW * (pr_ + 1)]
                            nc.tensor.matmul(
                                ps[0:PAIRW, 128 * k: 128 * k + PAIRW],
                                lhs, lhs,
                            )
                        # PSUM -> SBUF compacting copy (ScalarE sits by PSUM):
                        # gather the 8 pairs' useful [33, 32] blocks into a
                        # contiguous [33, 256] tile, then ONE DMA per parity
                        # remaps into the sample's stage row
                        if X_SKIP_EXTRACT:
                            continue
                        cp = scp.tile([P, PPT * 128], f32, tag="cp", name="cp")
                        nc.scalar.copy(
                            cp[0:PAIRW, :].rearrange(
                                "p (k c) -> p k c", c=128)[:, :, 0:PAIRW],
                            ps[0:PAIRW, :].rearrange(
                                "p (k c) -> p k c", c=128)[:, :, 0:PAIRW],
                        )
                        bi = (blk - blk0) * 2 + pt
                        for s in range(2):
                            pa = 64 * s + bi
                            dstv = stage[pa:pa + 1, :].rearrange(
                                "p (i slj) -> p i slj", i=AUGW)
                            srcv = cp[AUGW * s:AUGW * (s + 1), :].rearrange(
                                "p (sl c) -> p sl c", c=128
                            )[:, :, AUGW * s: AUGW * s + NT]
                            nc.sync.dma_start(dstv, srcv)
                tg = g_t[g]
                # stage per-partition layout: [i'(33), sl(8), j(32)]
                st2 = stage[:].rearrange("p (i sl j) -> p sl i j",
                                         i=AUGW, sl=NS, j=NT)
                nc.vector.tensor_copy(
                    tg["hht"][:].rearrange("p (sl a b) -> p sl a b",
                                           a=NT, b=NT),
                    st2[:, :, 0:NT, :],
                )
                nc.vector.tensor_copy(
                    tg["yh"][:].rearrange("p (sl j) -> p sl j", j=NT)
                    .unsqueeze(2),
                    st2[:, :, NT:NT + 1, :],
                )
                # sig2 broadcast tile (fp16) for this group
                nc.vector.tensor_copy(
                    tg["s2t"][:].rearrange("p (q c) -> p q c", c=ST),
                    sig2f[:, g * NS:(g + 1) * NS].unsqueeze(2)
                    .broadcast_to([P, NS, ST]),
                )
                nc.vector.memset(tg["Ga"][:], 0.0)

            def iteration(g, t):
                tg = g_t[g]
                Gc = tg["Ga"] if t % 2 == 0 else tg["Gb"]
                Gn = tg["Gb"] if t % 2 == 0 else tg["Ga"]
                e16, v16, em, exy = tg["e16"], tg["v16"], tg["em"], tg["exy"]
                ed, gq, z, t2 = tg["ed"], tg["gq"], tg["z"], tg["t2"]
                sa, sm, S, w = tg["sa"], tg["sm"], tg["S"], tg["w"]
                R, xt16, xhh = tg["R"], tg["xt16"], tg["xhh"]
                pr, q2 = tg["pr"], tg["q2"]

                s_t = float(s_sm[t])
                dta = float(d_vals[t] * ta_abs[t])
                d_t = float(d_vals[t])

                nc.scalar.activation(e16[:], Gc[:], ACTF.Exp, scale=s_t)
                if la_term is not None:
                    nc.vector.tensor_mul(e16[:], e16[:], la_term[:])
                nc.scalar.activation(v16[:], Gc[:], ACTF.Exp, scale=-1.0)

                nc.vector.tensor_mul(em[:], e16[:], m16[:])
                e_v = e16[:].rearrange("p (q i m) -> p q i m", i=NT, m=M)
                em_v = em[:].rearrange("p (q i m) -> p q i m", i=NT, m=M)
                sa_v = sa[:].rearrange("p (q i h) -> p q i h", i=NT, h=2)
                sm_v = sm[:].rearrange("p (q i h) -> p q i h", i=NT, h=2)
                nc.vector.tensor_add(sa_v, e_v[:, :, :, 0:2], e_v[:, :, :, 2:4])
                nc.vector.tensor_add(sm_v, em_v[:, :, :, 0:2], em_v[:, :, :, 2:4])
                sa_f = sa[:].rearrange("p (k h) -> p k h", h=2)
                sm_f = sm[:].rearrange("p (k h) -> p k h", h=2)
                nc.vector.tensor_add(S[:].unsqueeze(2), sa_f[:, :, 0:1], sa_f[:, :, 1:2])
                nc.vector.tensor_add(w[:].unsqueeze(2), sm_f[:, :, 0:1], sm_f[:, :, 1:2])
                _recip(nc, R[:], S[:])
                nc.vector.tensor_mul(xt16[:], w[:], R[:])

                # whale: xHH[q,j] = sum_i HH[q,j,i] * xt[q,i]
                xt_v = xt16[:].rearrange("p (q i) -> p q i", i=NT)
                hht_v = tg["hht"][:].rearrange("p (q j i) -> p q j i", j=NT, i=NT)
                tmp_v = tmp[:].rearrange("p (q j i) -> p q j i", j=NT, i=NT)
                if X_WHALE_UNIT:
                    nc.vector.tensor_mul(tmp[:], tg["hht"][:], tg["hht"][:])
                else:
                    nc.vector.tensor_mul(
                        tmp_v, hht_v,
                        xt_v.unsqueeze(2).broadcast_to([P, NS, NT, NT])
                    )
                r1_v = r1[:].rearrange("p (q j i) -> p q j i", j=NT, i=16)
                nc.vector.tensor_add(r1_v, tmp_v[:, :, :, 0:16], tmp_v[:, :, :, 16:32])
                r2_v = r2[:].rearrange("p (q j i) -> p q j i", j=NT, i=8)
                eng2 = nc.gpsimd if X_GPSIMD2 else nc.vector
                eng2.tensor_add(r2_v, r1_v[:, :, :, 0:8], r1_v[:, :, :, 8:16])
                r3_v = r3[:].rearrange("p (q j i) -> p q j i", j=NT, i=4)
                nc.vector.tensor_add(r3_v, r2_v[:, :, :, 0:4], r2_v[:, :, :, 4:8])
                r4_v = r4[:].rearrange("p (q j i) -> p q j i", j=NT, i=2)
                nc.vector.tensor_add(r4_v, r3_v[:, :, :, 0:2], r3_v[:, :, :, 2:4])
                r4_f = r4[:].rearrange("p (k h) -> p k h", h=2)
                nc.vector.tensor_add(xhh[:].unsqueeze(2), r4_f[:, :, 0:1], r4_f[:, :, 1:2])

                nc.vector.tensor_tensor(pr[:], xhh[:], tg["yh"][:], op=AL.subtract)
                nc.vector.scalar_tensor_tensor(
                    q2[:], pr[:], dta, R[:], op0=AL.mult, op1=AL.mult
                )

                eng1 = nc.gpsimd if X_GPSIMD else nc.vector
                eng1.tensor_mul(
                    exy[:].rearrange("p (q i m) -> p q i m", i=NT, m=M),
                    e_v,
                    xt_v.unsqueeze(3).broadcast_to([P, NS, NT, M]),
                )
                (nc.gpsimd if X_GPSIMD2 else nc.vector).tensor_tensor(
                    ed[:], em[:], exy[:], op=AL.subtract)
                eng1.tensor_mul(
                    gq[:].rearrange("p (q i m) -> p q i m", i=NT, m=M),
                    ed[:].rearrange("p (q i m) -> p q i m", i=NT, m=M),
                    q2[:].rearrange("p (q i) -> p q i", i=NT)
                    .unsqueeze(3).broadcast_to([P, NS, NT, M]),
                )
                if X_GPSIMD4:
                    nc.gpsimd.tensor_mul(z[:], v16[:], tg["s2t"][:])
                    nc.gpsimd.tensor_tensor(z[:], z[:], tg["s2t"][:],
                                            op=AL.subtract)
                else:
                    nc.vector.scalar_tensor_tensor(
                        z[:], v16[:], -1.0, tg["s2t"][:],
                        op0=AL.add, op1=AL.mult)
                (nc.gpsimd if X_GPSIMD3 else nc.vector).scalar_tensor_tensor(
                    t2[:], z[:], d_t, gq[:], op0=AL.mult, op1=AL.subtract
                )
                nc.vector.tensor_add(Gn[:], Gc[:], t2[:])

            def output(g):
                tg = g_t[g]
                Gfin = tg["Ga"] if NITER % 2 == 0 else tg["Gb"]
                ef = pp.tile([P, NS * ST], f32, tag="ef", name="ef")
                nc.scalar.activation(ef[:], Gfin[:], ACTF.Exp, scale=float(s_fin))
                if la_term is not None:
                    nc.vector.tensor_mul(ef[:], ef[:], la_term[:])
                emf = pp.tile([P, NS * ST], f32, tag="emf", name="emf")
                nc.vector.tensor_mul(emf[:], ef[:], m16[:])
                saf = pp.tile([P, NS * NT * 2], f32, tag="saf", name="saf")
                smf = pp.tile([P, NS * NT * 2], f32, tag="smf", name="smf")
                ef_v = ef[:].rearrange("p (q i m) -> p q i m", i=NT, m=M)
                emf_v = emf[:].rearrange("p (q i m) -> p q i m", i=NT, m=M)
                nc.vector.tensor_add(
                    saf[:].rearrange("p (q i h) -> p q i h", i=NT, h=2),
                    ef_v[:, :, :, 0:2], ef_v[:, :, :, 2:4])
                nc.vector.tensor_add(
                    smf[:].rearrange("p (q i h) -> p q i h", i=NT, h=2),
                    emf_v[:, :, :, 0:2], emf_v[:, :, :, 2:4])
                Sf = pp.tile([P, NS * NT], f32, tag="Sf", name="Sf")
                wf = pp.tile([P, NS * NT], f32, tag="wf", name="wf")
                saf_f = saf[:].rearrange("p (k h) -> p k h", h=2)
                smf_f = smf[:].rearrange("p (k h) -> p k h", h=2)
                nc.vector.tensor_add(Sf[:].unsqueeze(2), saf_f[:, :, 0:1], saf_f[:, :, 1:2])
                nc.vector.tensor_add(wf[:].unsqueeze(2), smf_f[:, :, 0:1], smf_f[:, :, 1:2])
                Rf = pp.tile([P, NS * NT], f32, tag="Rf", name="Rf")
                _recip(nc, Rf[:], Sf[:])
                ftb = pp.tile([P, NS * ST], f32, tag="ftb", name="ftb")
                nc.vector.tensor_mul(
                    ftb[:].rearrange("p (q i m) -> p q i m", i=NT, m=M),
                    ef_v,
                    Rf[:].rearrange("p (q i) -> p q i", i=NT)
                    .unsqueeze(3).broadcast_to([P, NS, NT, M]),
                )
                xtb = pp.tile([P, NS * NT], f32, tag="xtb", name="xtb")
                nc.vector.tensor_mul(xtb[:], wf[:], Rf[:])
                dst_ft = ft_d[g * GS:(g + 1) * GS].rearrange(
                    "(bi sl s) i m -> s bi sl i m", sl=NS, s=2)
                nc.sync.dma_start(
                    dst_ft, ftb[:].rearrange("p (q i m) -> p q i m", i=NT, m=M))
                dst_xt = xt_d[g * GS:(g + 1) * GS].rearrange(
                    "(bi sl s) i -> s bi sl i", sl=NS, s=2)
                nc.sync.dma_start(
                    dst_xt, xtb[:].rearrange("p (q i) -> p q i", i=NT))

            for g in range(NGROUP):
                if X_SKIP_PRE:
                    nc.vector.memset(g_t[g]["Ga"][:], 0.0)
                else:
                    precompute(g)
            for t in range(NITER):
                for g in range(NGROUP):
                    iteration(g, t)
            for g in range(NGROUP):
                output(g)

    nc.compile()
    return nc


def _prepare_scalars(m, alpha, taui, delta):
    m_vals = np.asarray(m, np.float64)
    alpha = np.asarray(alpha, np.float64)
    taui_abs = np.abs(np.asarray(taui, np.float64))
    d_vals = np.asarray(delta, np.float64)
    s_sm = taui_abs[:NITER].copy()
    s_sm[0] = 1.0
    s_fin = taui_abs[NITER]
    uniform = np.allclose(alpha, alpha[0])
    if not uniform:
        scales = np.concatenate([s_sm, [s_fin]])
        if not np.allclose(scales, scales[0]):
            raise NotImplementedError(
                "non-uniform alpha with varying softmax scales not supported")
    return m_vals, s_sm, taui_abs[:NITER], d_vals, s_fin, uniform, alpha


def kernel(yt, Ht, sigmat0, m, alpha, taui, delta):
    yt = np.ascontiguousarray(np.asarray(yt, np.float32))
    Ht = np.ascontiguousarray(np.asarray(Ht, np.float32))
    sigmat0 = np.ascontiguousarray(np.asarray(sigmat0, np.float32))
    (m_vals, s_sm, ta_abs, d_vals, s_fin, uniform, alpha_v) = _prepare_scalars(
        m, alpha, taui, delta)

    # la_term support (non-uniform alpha) is built inside _build; for the
    # uniform case softmax(la + G) == softmax(G) so it folds away entirely.
    if not uniform:
        raise NotImplementedError("non-uniform alpha path not wired up")

    key = (tuple(m_vals), tuple(s_sm), tuple(ta_abs), tuple(d_vals), float(s_fin))
    if key not in _CACHE:
        _CACHE[key] = _build((m_vals, s_sm, ta_abs, d_vals, s_fin, None))
    nc = _CACHE[key]

    in_maps = []
    for c in range(NCORES):
        sl = slice(c * BS, (c + 1) * BS)
        in_maps.append({
            "ht": Ht[sl],
            "yt": yt[sl],
            "sg": sigmat0[sl],
        })
    res = bass_utils.run_bass_kernel_spmd(nc, in_maps, core_ids=list(range(NCORES)))
    ft = np.concatenate([res.results[c]["ft"] for c in range(NCORES)], axis=0)
    xt = np.concatenate([res.results[c]["xt"] for c in range(NCORES)], axis=0)
    return ft.astype(np.float32), xt.astype(np.float32)


if __name__ == "__main__":
    d = np.load("/tmp/inputs.npz")
    ft, xt = kernel(**{k: d[k] for k in d.files})
    print(ft.shape, xt.shape, ft.dtype)


# revision 27
# speedup vs baseline: 1.4184x; 1.4184x over previous
"""CMDNet detector kernel for 8 Trainium2 NeuronCores.

Contract: kernel(**inputs) takes the FULL inputs of reference.setup_inputs()
(yt [16384,64], Ht [16384,64,32], sigmat0 [16384], m [4], alpha [4],
taui [65], delta [64]) and returns (ft [16384,32,4], xt [16384,32]) like
reference.reference().

Strategy (pure data parallel over batch):
 - shard batch over 8 cores (2048 samples each), identical SPMD program
 - per core: 2 groups x (128 partitions x 8 slots) sample layout
 - precompute per-sample HH = Ht^T Ht and yH = yt^T Ht on the TensorEngine
   via block-diagonal packing of the augmented matrix [Ht | yt] (2 samples
   per 128x66 matmul); extract PSUM -> per-sample SBUF layout with DMAs
 - 64 gradient-descent iterations run on Vector/Scalar engines in fp16
   (validated: end-to-end fp16 error ~5e-4 vs fp32 reference)
 - tiny per-iteration scalars (taui/delta + alpha uniformity) are read on
   the host and baked into the program as immediates
"""

import sys

sys.path.insert(0, "/opt/trn_rl_repo")

import numpy as np

import concourse.bass as bass  # noqa: F401  (bass types used implicitly)
import concourse.bacc as bacc
import concourse.mybir as mybir
import concourse.tile as tile
from concourse import bass_utils
from concourse.dve_ops import RECIPROCAL_APPROX_FAST, RECIP_APPROX_FAST_CONSTS

f16 = mybir.dt.float16
f32 = mybir.dt.float32
AL = mybir.AluOpType
ACTF = mybir.ActivationFunctionType

B, NR, NT, M, NITER = 16384, 64, 32, 4, 64
NCORES = 8
BS = B // NCORES          # samples per core
P = 128                   # SBUF partitions
NGROUP = 2                # groups per core
GS = BS // NGROUP         # samples per group (1024)
NS = GS // P              # slots per partition per group (8)
ST = NT * M               # state elems per sample (128)
HHW = NT * NT             # 1024
AUGW = NT + 1             # 33  (Ht cols + yt col)
PAIRW = 2 * AUGW          # 66  (block-diag pair width)
PAIRS = BS // 2           # 1024 pairs per core
PBLK = 16                 # pairs per input staging block
NBLK = PAIRS // PBLK      # 64 blocks
PPT = 8                   # pairs per PSUM tile (two banks, 128-elem pair pitch)

_CACHE = {}

# experiment flags (timing A/B via TimelineSim)
X_WHALE_UNIT = False   # whale mul with unit-stride in1 (wrong results; timing only)
X_SKIP_PRE = False     # skip gram precompute (wrong results; timing only)
X_GPSIMD = True        # offload exy/gq to gpsimd
X_GPSIMD2 = False      # also offload red2 + ed to gpsimd (model: worse)
X_GPSIMD3 = False      # t2 STT on gpsimd: opcode invalid on HW
X_G16 = True           # keep G state in fp16 (2x-mode updates)
X_GPSIMD4 = True       # z on gpsimd as two tensor_tensor ops
X_SKIP_EXTRACT = False # skip psum->stage extraction dmas (timing only)
X_SKIP_MM = False      # skip gram matmuls (timing only)
X_SKIP_INDMA = False   # skip ht/yt input dmas (timing only)


def _recip(nc, out_ap, in_ap):
    nc.vector._custom_dve(
        RECIPROCAL_APPROX_FAST,
        out=out_ap,
        in0=in_ap,
        s0=RECIP_APPROX_FAST_CONSTS["s0"],
        s1=RECIP_APPROX_FAST_CONSTS["s1"],
        imm2=RECIP_APPROX_FAST_CONSTS["imm2"],
    )


def _build(scalars):
    """Build + compile the per-core SPMD program. `scalars` carries the host
    -read values baked into the instruction stream."""
    m_vals, s_sm, ta_abs, d_vals, s_fin, la_term = scalars

    nc = bacc.Bacc("TRN2", target_bir_lowering=False, debug=False,
                   num_devices=NCORES)
    ht_d = nc.dram_tensor("ht", [BS, NR, NT], f32, kind="ExternalInput").ap()
    yt_d = nc.dram_tensor("yt", [BS, NR], f32, kind="ExternalInput").ap()
    sg_d = nc.dram_tensor("sg", [BS], f32, kind="ExternalInput").ap()
    ft_d = nc.dram_tensor("ft", [BS, NT, M], f32, kind="ExternalOutput").ap()
    xt_d = nc.dram_tensor("xt", [BS, NT], f32, kind="ExternalOutput").ap()

    ht_pairs = ht_d.rearrange("(k s) r i -> k s r i", s=2)
    yt_pairs = yt_d.rearrange("(k s) r -> k s r", s=2)

    with tile.TileContext(nc) as tc:
        with (
            tc.tile_pool(name="pers", bufs=1) as pp,
            tc.tile_pool(name="psum", bufs=4, space="PSUM") as psp,
            tc.tile_pool(name="pscopy", bufs=2) as scp,
        ):
            # ---------------- shared constants ----------------
            m16 = pp.tile([P, NS * ST], f16, tag="m16", name="m16")
            for mm in range(M):
                v = m16[:].rearrange("p (r m) -> p r m", m=M)[:, :, mm:mm + 1]
                nc.vector.memset(v, float(m_vals[mm]))

            sgf = pp.tile([P, NGROUP * NS], f32, tag="sgf", name="sgf")
            # sample (g, bi, sl, s) -> partition 64*s+bi, col g*NS+sl
            sg_v = sg_d.rearrange("(g bi sl s) -> g s bi sl", g=NGROUP, sl=NS, s=2)
            for g in range(NGROUP):
                for s in range(2):
                    nc.sync.dma_start(
                        sgf[64 * s:64 * (s + 1), g * NS:(g + 1) * NS],
                        sg_v[g, s],
                    )
            sig2f = pp.tile([P, NGROUP * NS], f32, tag="sig2f", name="sig2f")
            nc.scalar.activation(sig2f[:], sgf[:], ACTF.Square)

            # per-group persistent tiles
            g_t = {}
            for g in range(NGROUP):
                g_t[g] = dict(
                    hht=pp.tile([P, NS * HHW], f16, tag=f"hht{g}", name=f"hht{g}"),
                    yh=pp.tile([P, NS * NT], f32, tag=f"yh{g}", name=f"yh{g}"),
                    s2t=pp.tile([P, NS * ST], f16, tag=f"s2t{g}", name=f"s2t{g}"),
                    Ga=pp.tile([P, NS * ST], f16 if X_G16 else f32,
                                tag=f"Ga{g}", name=f"Ga{g}"),
                    Gb=pp.tile([P, NS * ST], f16 if X_G16 else f32,
                                tag=f"Gb{g}", name=f"Gb{g}"),
                    eem=pp.tile([P, 2 * NS * ST], f16, tag=f"eem{g}", name=f"eem{g}"),
                    v16=pp.tile([P, NS * ST], f16, tag=f"v16{g}", name=f"v16{g}"),
                    exy=pp.tile([P, NS * ST], f16, tag=f"exy{g}", name=f"exy{g}"),
                    ed=pp.tile([P, NS * ST], f16, tag=f"ed{g}", name=f"ed{g}"),
                    gq=pp.tile([P, NS * ST], f16, tag=f"gq{g}", name=f"gq{g}"),
                    z=pp.tile([P, NS * ST], f16, tag=f"z{g}", name=f"z{g}"),
                    t2=pp.tile([P, NS * ST], f16, tag=f"t2{g}", name=f"t2{g}"),
                    sab=pp.tile([P, 2 * NS * NT * 2], f16, tag=f"sab{g}", name=f"sab{g}"),
                    Sw=pp.tile([P, 2 * NS * NT], f32, tag=f"Sw{g}", name=f"Sw{g}"),
                    R=pp.tile([P, NS * NT], f32, tag=f"R{g}", name=f"R{g}"),
                    xt16=pp.tile([P, NS * NT], f16, tag=f"xt16{g}", name=f"xt16{g}"),
                    xhh=pp.tile([P, NS * NT], f32, tag=f"xhh{g}", name=f"xhh{g}"),
                    pr=pp.tile([P, NS * NT], f32, tag=f"pr{g}", name=f"pr{g}"),
                    q2=pp.tile([P, NS * NT], f16, tag=f"q2{g}", name=f"q2{g}"),
                )
            # whale scratch shared between groups (SBUF budget)
            tmp = pp.tile([P, NS * HHW], f16, tag="tmp", name="tmp")
            r1 = pp.tile([P, NS * NT * 16], f16, tag="r1", name="r1")
            r2 = pp.tile([P, NS * NT * 8], f16, tag="r2", name="r2")
            r3 = pp.tile([P, NS * NT * 4], f16, tag="r3", name="r3")
            r4 = pp.tile([P, NS * NT * 2], f16, tag="r4", name="r4")

            # block staging tiles for gram inputs (block-diagonal [Ht|y])
            Tb = [pp.tile([P, PBLK * PAIRW], f32, tag=f"Tb{i}", name=f"Tb{i}") for i in range(2)]
            nc.vector.memset(Tb[0][:], 0.0)
            nc.vector.memset(Tb[1][:], 0.0)
            T16b = [pp.tile([P, PBLK * PAIRW], f16, tag=f"T16b{i}", name=f"T16b{i}") for i in range(2)]

            # ---------------- per-group precompute + iterations -----------
            def precompute(g):
                stage = pp.tile([P, NS * AUGW * NT], f32, tag="stage", name="stage")
                blk0 = g * (NBLK // NGROUP)
                for blk in range(blk0, blk0 + NBLK // NGROUP):
                    T = Tb[blk % 2]
                    T16 = T16b[blk % 2]
                    # load [Ht | yt] diag blocks for 16 pairs
                    for s in range(2) if not X_SKIP_INDMA else []:
                        dst = T[64 * s:64 * (s + 1), :].rearrange(
                            "p (k c) -> p k c", c=PAIRW)
                        nc.sync.dma_start(
                            dst[:, :, AUGW * s: AUGW * s + NT],
                            ht_pairs[blk * PBLK:(blk + 1) * PBLK, s]
                            .transpose([1, 0, 2]),
                        )
                        nc.sync.dma_start(
                            dst[:, :, AUGW * s + NT: AUGW * s + NT + 1],
                            yt_pairs[blk * PBLK:(blk + 1) * PBLK, s]
                            .transpose([1, 0]).unsqueeze(2),
                        )
                    nc.vector.tensor_copy(T16[:], T[:])
                    for pt in range(PBLK // PPT) if not X_SKIP_MM else []:
                        ps = psp.tile([P, PPT * 128], f32, tag="gram", name="gram")
                        for k in range(PPT):
                            pr_ = pt * PPT + k
                            lhs = T16[:, PAIRW * pr_: PAIRW * (pr_ + 1)]
                            nc.tensor.matmul(
                                ps[0:PAIRW, 128 * k: 128 * k + PAIRW],
                                lhs, lhs,
                            )
                        # PSUM -> SBUF compacting copy (ScalarE sits by PSUM):
                        # gather the 8 pairs' useful [33, 32] blocks into a
                        # contiguous [33, 256] tile, then ONE DMA per parity
                        # remaps into the sample's stage row
                        if X_SKIP_EXTRACT:
                            continue
                        cp = scp.tile([P, PPT * 128], f32, tag="cp", name="cp")
                        nc.scalar.copy(
                            cp[0:PAIRW, :].rearrange(
                                "p (k c) -> p k c", c=128)[:, :, 0:PAIRW],
                            ps[0:PAIRW, :].rearrange(
                                "p (k c) -> p k c", c=128)[:, :, 0:PAIRW],
                        )
                        bi = (blk - blk0) * 2 + pt
                        for s in range(2):
                            pa = 64 * s + bi
                            dstv = stage[pa:pa + 1, :].rearrange(
                                "p (i slj) -> p i slj", i=AUGW)
                            srcv = cp[AUGW * s:AUGW * (s + 1), :].rearrange(
                                "p (sl c) -> p sl c", c=128
                            )[:, :, AUGW * s: AUGW * s + NT]
                            nc.sync.dma_start(dstv, srcv)
                tg = g_t[g]
                # stage per-partition layout: [i'(33), sl(8), j(32)]
                st2 = stage[:].rearrange("p (i sl j) -> p sl i j",
                                         i=AUGW, sl=NS, j=NT)
                nc.vector.tensor_copy(
                    tg["hht"][:].rearrange("p (sl a b) -> p sl a b",
                                           a=NT, b=NT),
                    st2[:, :, 0:NT, :],
                )
                nc.vector.tensor_copy(
                    tg["yh"][:].rearrange("p (sl j) -> p sl j", j=NT)
                    .unsqueeze(2),
                    st2[:, :, NT:NT + 1, :],
                )
                # sig2 broadcast tile (fp16) for this group
                nc.vector.tensor_copy(
                    tg["s2t"][:].rearrange("p (q c) -> p q c", c=ST),
                    sig2f[:, g * NS:(g + 1) * NS].unsqueeze(2)
                    .broadcast_to([P, NS, ST]),
                )
                nc.vector.memset(tg["Ga"][:], 0.0)

            def iteration(g, t):
                tg = g_t[g]
                Gc = tg["Ga"] if t % 2 == 0 else tg["Gb"]
                Gn = tg["Gb"] if t % 2 == 0 else tg["Ga"]
                eem, v16, exy = tg["eem"], tg["v16"], tg["exy"]
                ed, gq, z, t2 = tg["ed"], tg["gq"], tg["z"], tg["t2"]
                sab, Sw = tg["sab"], tg["Sw"]
                R, xt16, xhh = tg["R"], tg["xt16"], tg["xhh"]
                pr, q2 = tg["pr"], tg["q2"]
                NSST = NS * ST
                e16 = eem[:, 0:NSST]
                em = eem[:, NSST:2 * NSST]
                S = Sw[:, 0:NS * NT]
                w = Sw[:, NS * NT:2 * NS * NT]

                s_t = float(s_sm[t])
                dta = float(d_vals[t] * ta_abs[t])
                d_t = float(d_vals[t])

                nc.scalar.activation(e16, Gc[:], ACTF.Exp, scale=s_t)
                if la_term is not None:
                    nc.vector.tensor_mul(e16, e16, la_term[:])
                nc.scalar.activation(v16[:], Gc[:], ACTF.Exp, scale=-1.0)

                nc.vector.tensor_mul(em, e16, m16[:])
                e_v = e16.rearrange("p (q i m) -> p q i m", i=NT, m=M)
                ee_v = eem[:].rearrange("p (hq i m) -> p hq i m", i=NT, m=M)
                sab_v = sab[:].rearrange("p (hq i h) -> p hq i h", i=NT, h=2)
                nc.vector.tensor_add(sab_v, ee_v[:, :, :, 0:2], ee_v[:, :, :, 2:4])
                sab_f = sab[:].rearrange("p (k h) -> p k h", h=2)
                nc.vector.tensor_add(Sw[:].unsqueeze(2), sab_f[:, :, 0:1], sab_f[:, :, 1:2])
                _recip(nc, R[:], S)
                nc.vector.tensor_mul(xt16[:], w, R[:])

                # whale: xHH[q,j] = sum_i HH[q,j,i] * xt[q,i]
                xt_v = xt16[:].rearrange("p (q i) -> p q i", i=NT)
                hht_v = tg["hht"][:].rearrange("p (q j i) -> p q j i", j=NT, i=NT)
                tmp_v = tmp[:].rearrange("p (q j i) -> p q j i", j=NT, i=NT)
                if X_WHALE_UNIT:
                    nc.vector.tensor_mul(tmp[:], tg["hht"][:], tg["hht"][:])
                else:
                    nc.vector.tensor_mul(
                        tmp_v, hht_v,
                        xt_v.unsqueeze(2).broadcast_to([P, NS, NT, NT])
                    )
                r1_v = r1[:].rearrange("p (q j i) -> p q j i", j=NT, i=16)
                nc.vector.tensor_add(r1_v, tmp_v[:, :, :, 0:16], tmp_v[:, :, :, 16:32])
                r2_v = r2[:].rearrange("p (q j i) -> p q j i", j=NT, i=8)
                eng2 = nc.gpsimd if X_GPSIMD2 else nc.vector
                eng2.tensor_add(r2_v, r1_v[:, :, :, 0:8], r1_v[:, :, :, 8:16])
                r3_v = r3[:].rearrange("p (q j i) -> p q j i", j=NT, i=4)
                nc.vector.tensor_add(r3_v, r2_v[:, :, :, 0:4], r2_v[:, :, :, 4:8])
                r4_v = r4[:].rearrange("p (q j i) -> p q j i", j=NT, i=2)
                nc.vector.tensor_add(r4_v, r3_v[:, :, :, 0:2], r3_v[:, :, :, 2:4])
                r4_f = r4[:].rearrange("p (k h) -> p k h", h=2)
                nc.vector.tensor_add(xhh[:].unsqueeze(2), r4_f[:, :, 0:1], r4_f[:, :, 1:2])

                nc.vector.tensor_tensor(pr[:], xhh[:], tg["yh"][:], op=AL.subtract)
                nc.vector.scalar_tensor_tensor(
                    q2[:], pr[:], dta, R[:], op0=AL.mult, op1=AL.mult
                )

                eng1 = nc.gpsimd if X_GPSIMD else nc.vector
                eng1.tensor_mul(
                    exy[:].rearrange("p (q i m) -> p q i m", i=NT, m=M),
                    e_v,
                    xt_v.unsqueeze(3).broadcast_to([P, NS, NT, M]),
                )
                (nc.gpsimd if X_GPSIMD2 else nc.vector).tensor_tensor(
                    ed[:], em, exy[:], op=AL.subtract)
                eng1.tensor_mul(
                    gq[:].rearrange("p (q i m) -> p q i m", i=NT, m=M),
                    ed[:].rearrange("p (q i m) -> p q i m", i=NT, m=M),
                    q2[:].rearrange("p (q i) -> p q i", i=NT)
                    .unsqueeze(3).broadcast_to([P, NS, NT, M]),
                )
                if X_GPSIMD4:
                    nc.gpsimd.tensor_mul(z[:], v16[:], tg["s2t"][:])
                    nc.gpsimd.tensor_tensor(z[:], z[:], tg["s2t"][:],
                                            op=AL.subtract)
                else:
                    nc.vector.scalar_tensor_tensor(
                        z[:], v16[:], -1.0, tg["s2t"][:],
                        op0=AL.add, op1=AL.mult)
                (nc.gpsimd if X_GPSIMD3 else nc.vector).scalar_tensor_tensor(
                    t2[:], z[:], d_t, gq[:], op0=AL.mult, op1=AL.subtract
                )
                nc.vector.tensor_add(Gn[:], Gc[:], t2[:])

            def output(g):
                tg = g_t[g]
                Gfin = tg["Ga"] if NITER % 2 == 0 else tg["Gb"]
                ef = pp.tile([P, NS * ST], f32, tag="ef", name="ef")
                nc.scalar.activation(ef[:], Gfin[:], ACTF.Exp, scale=float(s_fin))
                if la_term is not None:
                    nc.vector.tensor_mul(ef[:], ef[:], la_term[:])
                emf = pp.tile([P, NS * ST], f32, tag="emf", name="emf")
                nc.vector.tensor_mul(emf[:], ef[:], m16[:])
                saf = pp.tile([P, NS * NT * 2], f32, tag="saf", name="saf")
                smf = pp.tile([P, NS * NT * 2], f32, tag="smf", name="smf")
                ef_v = ef[:].rearrange("p (q i m) -> p q i m", i=NT, m=M)
                emf_v = emf[:].rearrange("p (q i m) -> p q i m", i=NT, m=M)
                nc.vector.tensor_add(
                    saf[:].rearrange("p (q i h) -> p q i h", i=NT, h=2),
                    ef_v[:, :, :, 0:2], ef_v[:, :, :, 2:4])
                nc.vector.tensor_add(
                    smf[:].rearrange("p (q i h) -> p q i h", i=NT, h=2),
                    emf_v[:, :, :, 0:2], emf_v[:, :, :, 2:4])
                Sf = pp.tile([P, NS * NT], f32, tag="Sf", name="Sf")
                wf = pp.tile([P, NS * NT], f32, tag="wf", name="wf")
                saf_f = saf[:].rearrange("p (k h) -> p k h", h=2)
                smf_f = smf[:].rearrange("p (k h) -> p k h", h=2)
                nc.vector.tensor_add(Sf[:].unsqueeze(2), saf_f[:, :, 0:1], saf_f[:, :, 1:2])
                nc.vector.tensor_add(wf[:].unsqueeze(2), smf_f[:, :, 0:1], smf_f[:, :, 1:2])
                Rf = pp.tile([P, NS * NT], f32, tag="Rf", name="Rf")
                _recip(nc, Rf[:], Sf[:])
                ftb = pp.tile([P, NS * ST], f32, tag="ftb", name="ftb")
                nc.vector.tensor_mul(
                    ftb[:].rearrange("p (q i m) -> p q i m", i=NT, m=M),
                    ef_v,
                    Rf[:].rearrange("p (q i) -> p q i", i=NT)
                    .unsqueeze(3).broadcast_to([P, NS, NT, M]),
                )
                xtb = pp.tile([P, NS * NT], f32, tag="xtb", name="xtb")
                nc.vector.tensor_mul(xtb[:], wf[:], Rf[:])
                dst_ft = ft_d[g * GS:(g + 1) * GS].rearrange(
                    "(bi sl s) i m -> s bi sl i m", sl=NS, s=2)
                nc.sync.dma_start(
                    dst_ft, ftb[:].rearrange("p (q i m) -> p q i m", i=NT, m=M))
                dst_xt = xt_d[g * GS:(g + 1) * GS].rearrange(
                    "(bi sl s) i -> s bi sl i", sl=NS, s=2)
                nc.sync.dma_start(
                    dst_xt, xtb[:].rearrange("p (q i) -> p q i", i=NT))

            for g in range(NGROUP):
                if X_SKIP_PRE:
                    nc.vector.memset(g_t[g]["Ga"][:], 0.0)
                else:
                    precompute(g)
            for t in range(NITER):
                for g in range(NGROUP):
                    iteration(g, t)
            for g in range(NGROUP):
                output(g)

    nc.compile()
    return nc


def _prepare_scalars(m, alpha, taui, delta):
    m_vals = np.asarray(m, np.float64)
    alpha = np.asarray(alpha, np.float64)
    taui_abs = np.abs(np.asarray(taui, np.float64))
    d_vals = np.asarray(delta, np.float64)
    s_sm = taui_abs[:NITER].copy()
    s_sm[0] = 1.0
    s_fin = taui_abs[NITER]
    uniform = np.allclose(alpha, alpha[0])
    if not uniform:
        scales = np.concatenate([s_sm, [s_fin]])
        if not np.allclose(scales, scales[0]):
            raise NotImplementedError(
                "non-uniform alpha with varying softmax scales not supported")
    return m_vals, s_sm, taui_abs[:NITER], d_vals, s_fin, uniform, alpha


def kernel(yt, Ht, sigmat0, m, alpha, taui, delta):
    yt = np.ascontiguousarray(np.asarray(yt, np.float32))
    Ht = np.ascontiguousarray(np.asarray(Ht, np.float32))
    sigmat0 = np.ascontiguousarray(np.asarray(sigmat0, np.float32))
    (m_vals, s_sm, ta_abs, d_vals, s_fin, uniform, alpha_v) = _prepare_scalars(
        m, alpha, taui, delta)

    # la_term support (non-uniform alpha) is built inside _build; for the
    # uniform case softmax(la + G) == softmax(G) so it folds away entirely.
    if not uniform:
        raise NotImplementedError("non-uniform alpha path not wired up")

    key = (tuple(m_vals), tuple(s_sm), tuple(ta_abs), tuple(d_vals), float(s_fin))
    if key not in _CACHE:
        _CACHE[key] = _build((m_vals, s_sm, ta_abs, d_vals, s_fin, None))
    nc = _CACHE[key]

    in_maps = []
    for c in range(NCORES):
        sl = slice(c * BS, (c + 1) * BS)
        in_maps.append({
            "ht": Ht[sl],
            "yt": yt[sl],
            "sg": sigmat0[sl],
        })
    res = bass_utils.run_bass_kernel_spmd(nc, in_maps, core_ids=list(range(NCORES)))
    ft = np.concatenate([res.results[c]["ft"] for c in range(NCORES)], axis=0)
    xt = np.concatenate([res.results[c]["xt"] for c in range(NCORES)], axis=0)
    return ft.astype(np.float32), xt.astype(np.float32)


if __name__ == "__main__":
    d = np.load("/tmp/inputs.npz")
    ft, xt = kernel(**{k: d[k] for k in d.files})
    print(ft.shape, xt.shape, ft.dtype)
